# revision 1
# baseline (speedup 1.0000x reference)
"""F81 supervised loss (Felsenstein pruning on balanced 64-leaf tree) on 8 TRN2 cores.

Layout: sites on partitions. Per core 16384 sites = NCHUNK chunks of 128
partitions x C site-groups. Leaves stored in bit-reversed order so every
tree level's sibling merge is (first half) * (second half), contiguous.
"""

import numpy as np
import concourse.bass as bass  # noqa: F401
import concourse.tile as tile
from concourse import bacc, mybir
from concourse.bass_utils import run_bass_kernel_spmd

F32 = mybir.dt.float32
OP = mybir.AluOpType
AF = mybir.ActivationFunctionType
AX = mybir.AxisListType

N_CORES = 8
B, L, S = 16, 8192, 64
NST = 4
MU = 1.0
EPS = 1e-12
SITES = B * L
CORE_SITES = SITES // N_CORES  # 16384
C = 8                          # site-groups per chunk (free-dim batching)
CHUNK_SITES = 128 * C          # 1024
NCHUNK = CORE_SITES // CHUNK_SITES  # 16

# levels: k=0..5, n_k nodes input, merge to n_k/2
LEVEL_N = [64, 32, 16, 8, 4, 2]
# offsets of level slices within the 504-long (x4 state-repeated) a vector
A_OFF = [0, 256, 384, 448, 480, 496]
# offsets of log slots within the 63-per-group log buffer
LOG_OFF = [0, 32, 48, 56, 60, 62]


def _bitrev(i, bits):
    r = 0
    for _ in range(bits):
        r = (r << 1) | (i & 1)
        i >>= 1
    return r


def _build_branch_vec(branch_lengths):
    """[504] f32: per level, t at bitrev node order, each repeated x4."""
    out = np.empty(504, np.float32)
    off_in = 0
    for k, n in enumerate(LEVEL_N):
        t = branch_lengths[off_in:off_in + n]
        off_in += n
        bits = 6 - k
        perm = [_bitrev(p, bits) for p in range(n)]
        seg = np.repeat(t[perm], NST)
        out[A_OFF[k]:A_OFF[k] + 4 * n] = seg
    return out


def _tree(nc, pool, pi, eq4, gap, a_all, logbuf, ll):
    """Emit one Felsenstein tree for per-site frequencies `pi` [128, C*4].

    eq4: [128,C,64,4] one-hot view, gap: [128, C*64], a_all: [128,504],
    logbuf: [128, C*63] tile, ll out: [128, C] tile.
    """
    pib = pi[:].rearrange("p (c s) -> p c s", c=C)

    # ---- level 0 (closed form on eq/gap) ----
    n = 64
    tmp = pool.tile([128, C * 256], F32, tag="tmp", bufs=2)
    tmp4 = tmp[:].rearrange("p (c n s) -> p c n s", c=C, s=NST)
    nc.vector.tensor_mul(tmp4, eq4, pib.unsqueeze(2).broadcast_to([128, C, n, NST]))
    dot = pool.tile([128, C * 64], F32, tag="dot", bufs=2)
    dot3 = dot[:].rearrange("p (c n) -> p c n", c=C)
    nc.vector.tensor_reduce(dot3, tmp4, axis=AX.X, op=OP.add)
    dg = pool.tile([128, C * 64], F32, tag="dg", bufs=2)
    nc.vector.tensor_add(dg[:], dot[:], gap[:])
    dot0b = dot3.unsqueeze(3).broadcast_to([128, C, n, NST])
    u = pool.tile([128, C * 256], F32, tag="u", bufs=2)
    u4 = u[:].rearrange("p (c n s) -> p c n s", c=C, s=NST)
    nc.vector.tensor_sub(u4, eq4, dot0b)
    a0 = a_all[:][:, A_OFF[0]:A_OFF[0] + 256].rearrange("p (n s) -> p n s", s=NST)
    v = pool.tile([128, C * 256], F32, tag="v", bufs=2)
    v4 = v[:].rearrange("p (c n s) -> p c n s", c=C, s=NST)
    nc.vector.tensor_mul(v4, u4, a0.unsqueeze(1).broadcast_to([128, C, n, NST]))
    m = pool.tile([128, C * 256], F32, tag="m", bufs=2)
    m4 = m[:].rearrange("p (c n s) -> p c n s", c=C, s=NST)
    dgb = dg[:].rearrange("p (c n) -> p c n", c=C).unsqueeze(3).broadcast_to(
        [128, C, n, NST])
    nc.vector.tensor_add(m4, v4, dgb)

    Lc = None
    for k in range(6):
        n = LEVEL_N[k]
        if k > 0:
            # generic message from current Lc [128, C, n, 4]
            Lc4 = Lc[:].rearrange("p (c n s) -> p c n s", c=C, s=NST)
            tmp = pool.tile([128, C * 4 * n], F32, tag="tmp", bufs=2)
            tmp4 = tmp[:].rearrange("p (c n s) -> p c n s", c=C, s=NST)
            nc.vector.tensor_mul(
                tmp4, Lc4, pib.unsqueeze(2).broadcast_to([128, C, n, NST]))
            dot = pool.tile([128, C * n], F32, tag="dot", bufs=2)
            dot3 = dot[:].rearrange("p (c n) -> p c n", c=C)
            nc.vector.tensor_reduce(dot3, tmp4, axis=AX.X, op=OP.add)
            dotb = dot3.unsqueeze(3).broadcast_to([128, C, n, NST])
            u = pool.tile([128, C * 4 * n], F32, tag="u", bufs=2)
            u4 = u[:].rearrange("p (c n s) -> p c n s", c=C, s=NST)
            nc.vector.tensor_sub(u4, Lc4, dotb)
            ak = a_all[:][:, A_OFF[k]:A_OFF[k] + 4 * n].rearrange(
                "p (n s) -> p n s", s=NST)
            v = pool.tile([128, C * 4 * n], F32, tag="v", bufs=2)
            v4 = v[:].rearrange("p (c n s) -> p c n s", c=C, s=NST)
            nc.vector.tensor_mul(v4, u4, ak.unsqueeze(1).broadcast_to([128, C, n, NST]))
            m = pool.tile([128, C * 4 * n], F32, tag="m", bufs=2)
            m4 = m[:].rearrange("p (c n s) -> p c n s", c=C, s=NST)
            nc.vector.tensor_add(m4, v4, dotb)

        # merge contiguous halves -> [128, C, n/2, 4]
        nn = n // 2
        Lnew = pool.tile([128, C * 4 * nn], F32, tag=f"L{k}", bufs=2)
        Ln4 = Lnew[:].rearrange("p (c n s) -> p c n s", c=C, s=NST)
        nc.vector.tensor_mul(Ln4, m4[:, :, 0:nn, :], m4[:, :, nn:n, :])

        # rescale: s = max(sum_states, EPS); logbuf slot; Lc = Lnew / s
        s = pool.tile([128, C * nn], F32, tag="s", bufs=2)
        s3 = s[:].rearrange("p (c n) -> p c n", c=C)
        nc.vector.tensor_reduce(s3, Ln4, axis=AX.X, op=OP.add)
        nc.vector.tensor_scalar_max(s[:], s[:], EPS)
        lb3 = logbuf[:].rearrange("p (c n) -> p c n", c=C)
        nc.scalar.activation(lb3[:, :, LOG_OFF[k]:LOG_OFF[k] + nn], s3, AF.Ln)
        r = pool.tile([128, C * nn], F32, tag="r", bufs=2)
        nc.vector.reciprocal(r[:], s[:])
        Lc = pool.tile([128, C * 4 * nn], F32, tag=f"Ln{k}", bufs=2)
        Lc4 = Lc[:].rearrange("p (c n s) -> p c n s", c=C, s=NST)
        rb = r[:].rearrange("p (c n) -> p c n", c=C).unsqueeze(3).broadcast_to(
            [128, C, nn, NST])
        nc.vector.tensor_mul(Lc4, Ln4, rb)
        m4 = None

    # root: ll = log(max(pi . Lc, EPS)) + sum(logbuf)
    Lc4 = Lc[:].rearrange("p (c n s) -> p c n s", c=C, s=NST)
    rtmp = pool.tile([128, C * 4], F32, tag="rtmp", bufs=2)
    rtmp4 = rtmp[:].rearrange("p (c n s) -> p c n s", c=C, s=NST)
    nc.vector.tensor_mul(rtmp4, Lc4, pib.unsqueeze(2).broadcast_to([128, C, 1, NST]))
    rdot = pool.tile([128, C], F32, tag="rdot", bufs=2)
    rdot3 = rdot[:].rearrange("p (c n) -> p c n", c=C)
    nc.vector.tensor_reduce(rdot3, rtmp4, axis=AX.X, op=OP.add)
    nc.vector.tensor_scalar_max(rdot[:], rdot[:], EPS)
    lr = pool.tile([128, C], F32, tag="lr", bufs=2)
    nc.scalar.activation(lr[:], rdot[:], AF.Ln)
    lsum = pool.tile([128, C], F32, tag="lsum", bufs=2)
    nc.vector.tensor_reduce(
        lsum[:].rearrange("p c -> p c", c=C),
        logbuf[:].rearrange("p (c n) -> p c n", c=C), axis=AX.X, op=OP.add)
    nc.vector.tensor_add(ll[:], lr[:], lsum[:])


def _build_program():
    nc = bacc.Bacc("TRN2", target_bir_lowering=False, debug=False,
                   num_devices=N_CORES)
    cod_d = nc.dram_tensor("cod", [CORE_SITES, S], F32, kind="ExternalInput")
    lg_d = nc.dram_tensor("lg", [CORE_SITES, NST], F32, kind="ExternalInput")
    pt_d = nc.dram_tensor("pt", [CORE_SITES, NST], F32, kind="ExternalInput")
    mk_d = nc.dram_tensor("mk", [CORE_SITES], F32, kind="ExternalInput")
    bl_d = nc.dram_tensor("bl", [504], F32, kind="ExternalInput")
    po_d = nc.dram_tensor("po", [128, 2], F32, kind="ExternalOutput")

    cod_r = cod_d.ap().rearrange("(k p c) l -> k p (c l)", k=NCHUNK, p=128)
    lg_r = lg_d.ap().rearrange("(k p c) s -> k p (c s)", k=NCHUNK, p=128)
    pt_r = pt_d.ap().rearrange("(k p c) s -> k p (c s)", k=NCHUNK, p=128)
    mk_r = mk_d.ap().rearrange("(k p c) -> k p c", k=NCHUNK, p=128)

    with tile.TileContext(nc) as tc:
        with (tc.tile_pool(name="const", bufs=1) as cpool,
              tc.tile_pool(name="work", bufs=2) as pool):
            a_all = cpool.tile([128, 504], F32)
            nc.sync.dma_start(a_all[:], bl_d.ap().unsqueeze(0).broadcast_to([128, 504]))
            nc.scalar.activation(a_all[:], a_all[:], AF.Exp, scale=-MU)
            acc = cpool.tile([128, 1], F32)
            nc.vector.memset(acc[:], 0.0)
            cnt = cpool.tile([128, 1], F32)
            nc.vector.memset(cnt[:], 0.0)

            for kc in range(NCHUNK):
                cod = pool.tile([128, C * S], F32, tag="cod", bufs=2)
                nc.sync.dma_start(cod[:], cod_r[kc])
                lg = pool.tile([128, C * NST], F32, tag="lg", bufs=2)
                nc.sync.dma_start(lg[:], lg_r[kc])
                pt = pool.tile([128, C * NST], F32, tag="pt", bufs=2)
                nc.sync.dma_start(pt[:], pt_r[kc])
                mk = pool.tile([128, C], F32, tag="mk", bufs=2)
                nc.sync.dma_start(mk[:], mk_r[kc])

                # one-hot + gap
                eq = pool.tile([128, C * S * NST], F32, tag="eq", bufs=2)
                eq4 = eq[:].rearrange("p (c l s) -> p c l s", c=C, s=NST)
                cod3 = cod[:].rearrange("p (c l) -> p c l", c=C)
                for c in range(NST):
                    nc.vector.tensor_scalar(eq4[:, :, :, c], cod3, float(c),
                                            None, OP.is_equal)
                gap = pool.tile([128, C * S], F32, tag="gap", bufs=2)
                nc.vector.tensor_scalar(gap[:], cod[:], 4.0, None, OP.is_ge)

                # pi_pred = softmax(logits)
                lg3 = lg[:].rearrange("p (c s) -> p c s", c=C)
                mx = pool.tile([128, C], F32, tag="mx", bufs=2)
                mx3 = mx[:].rearrange("p (c n) -> p c n", c=C)
                nc.vector.tensor_reduce(mx3, lg3, axis=AX.X, op=OP.max)
                lz = pool.tile([128, C * NST], F32, tag="lz", bufs=2)
                lz3 = lz[:].rearrange("p (c s) -> p c s", c=C)
                nc.vector.tensor_sub(lz3, lg3, mx3.broadcast_to([128, C, NST]))
                ex = pool.tile([128, C * NST], F32, tag="ex", bufs=2)
                nc.scalar.activation(ex[:], lz[:], AF.Exp)
                se = pool.tile([128, C], F32, tag="se", bufs=2)
                se3 = se[:].rearrange("p (c n) -> p c n", c=C)
                nc.vector.tensor_reduce(
                    se3, ex[:].rearrange("p (c s) -> p c s", c=C),
                    axis=AX.X, op=OP.add)
                re = pool.tile([128, C], F32, tag="re", bufs=2)
                nc.vector.reciprocal(re[:], se[:])
                pip = pool.tile([128, C * NST], F32, tag="pip", bufs=2)
                nc.vector.tensor_mul(
                    pip[:].rearrange("p (c s) -> p c s", c=C),
                    ex[:].rearrange("p (c s) -> p c s", c=C),
                    re[:].rearrange("p (c n) -> p c n", c=C).broadcast_to(
                        [128, C, NST]))

                # pi_true normalized
                ptc = pool.tile([128, C * NST], F32, tag="ptc", bufs=2)
                nc.vector.tensor_scalar_max(ptc[:], pt[:], EPS)
                se2 = pool.tile([128, C], F32, tag="se2", bufs=2)
                se23 = se2[:].rearrange("p (c n) -> p c n", c=C)
                nc.vector.tensor_reduce(
                    se23, ptc[:].rearrange("p (c s) -> p c s", c=C),
                    axis=AX.X, op=OP.add)
                re2 = pool.tile([128, C], F32, tag="re2", bufs=2)
                nc.vector.reciprocal(re2[:], se2[:])
                pit = pool.tile([128, C * NST], F32, tag="pit", bufs=2)
                nc.vector.tensor_mul(
                    pit[:].rearrange("p (c s) -> p c s", c=C),
                    ptc[:].rearrange("p (c s) -> p c s", c=C),
                    re2[:].rearrange("p (c n) -> p c n", c=C).broadcast_to(
                        [128, C, NST]))

                ll_p = pool.tile([128, C], F32, tag="ll_p", bufs=2)
                logbuf_p = pool.tile([128, C * 63], F32, tag="logbuf_p", bufs=2)
                _tree(nc, pool, pip, eq4, gap, a_all, logbuf_p, ll_p)
                ll_t = pool.tile([128, C], F32, tag="ll_t", bufs=2)
                logbuf_t = pool.tile([128, C * 63], F32, tag="logbuf_t", bufs=2)
                _tree(nc, pool, pit, eq4, gap, a_all, logbuf_t, ll_t)

                diff = pool.tile([128, C], F32, tag="diff", bufs=2)
                nc.vector.tensor_sub(diff[:], ll_t[:], ll_p[:])
                dm = pool.tile([128, C], F32, tag="dm", bufs=2)
                nc.vector.tensor_mul(dm[:], diff[:], mk[:])
                racc = pool.tile([128, 1], F32, tag="racc", bufs=2)
                nc.vector.tensor_reduce(racc[:], dm[:], axis=AX.X, op=OP.add)
                nc.vector.tensor_add(acc[:], acc[:], racc[:])
                rcnt = pool.tile([128, 1], F32, tag="rcnt", bufs=2)
                nc.vector.tensor_reduce(rcnt[:], mk[:], axis=AX.X, op=OP.add)
                nc.vector.tensor_add(cnt[:], cnt[:], rcnt[:])

            po = cpool.tile([128, 2], F32)
            nc.vector.tensor_copy(po[:, 0:1], acc[:])
            nc.vector.tensor_copy(po[:, 1:2], cnt[:])
            nc.sync.dma_start(po_d.ap()[:], po[:])

    nc.compile()
    return nc


_NC_CACHE = None


def _get_nc():
    global _NC_CACHE
    if _NC_CACHE is None:
        _NC_CACHE = _build_program()
    return _NC_CACHE


def kernel(logits, msa_codes, pi_true, valid_mask, branch_lengths):
    nc = _get_nc()
    perm = np.array([_bitrev(p, 6) for p in range(S)])
    cod = np.ascontiguousarray(
        msa_codes.reshape(SITES, S)[:, perm]).astype(np.float32)
    lg = np.ascontiguousarray(logits.reshape(SITES, NST)).astype(np.float32)
    pt = np.ascontiguousarray(pi_true.reshape(SITES, NST)).astype(np.float32)
    mk = valid_mask.reshape(SITES).astype(np.float32)
    bl = _build_branch_vec(np.asarray(branch_lengths, np.float32))

    in_maps = []
    for i in range(N_CORES):
        sl = slice(i * CORE_SITES, (i + 1) * CORE_SITES)
        in_maps.append(dict(cod=cod[sl], lg=lg[sl], pt=pt[sl], mk=mk[sl], bl=bl))
    res = run_bass_kernel_spmd(nc, in_maps, core_ids=list(range(N_CORES)))
    tot = 0.0
    n = 0.0
    for i in range(N_CORES):
        po = res.results[i]["po"]
        tot += float(po[:, 0].sum())
        n += float(po[:, 1].sum())
    return np.float32(tot / max(n, 1.0))


# revision 8
# speedup vs baseline: 250.8557x; 250.8557x over previous
"""F81 supervised loss (Felsenstein pruning, balanced 64-leaf tree) on 8 TRN2 cores.

v4: bf16 plane layout [group, state, node], bit-reversed leaves (merges =
contiguous half-products in DVE 2x mode). Algebra: w = L + beta*dot with
beta=(1-a)/a; all per-node a-factors become per-tree constants that cancel
in ll_true - ll_pred, so no per-level normalization is needed -- one
adaptive rescale at level 3 keeps bf16 in range. All Exp/Copy activation
work is batched upfront so the chunk loop only uses Ln (a single ACT
table set -> no table-reload thrash). Gap leaves handled in closed form
(dot += gap; q += gap) instead of materializing leaf conditionals.
"""

import numpy as np
import ml_dtypes
import concourse.bass as bass  # noqa: F401
import concourse.tile as tile
from concourse import bacc, mybir
from concourse.bass_utils import run_bass_kernel_spmd

F32 = mybir.dt.float32
BF16 = mybir.dt.bfloat16
OP = mybir.AluOpType
AF = mybir.ActivationFunctionType
AX = mybir.AxisListType

N_CORES = 8
B, L, S = 16, 8192, 64
NST = 4
MU = 1.0
EPS = 1e-12
TINY = 1e-35
SITES = B * L
CORE_SITES = SITES // N_CORES   # 16384
C = 16                          # site-groups per chunk
CHUNK_SITES = 128 * C           # 2048
NCHUNK = CORE_SITES // CHUNK_SITES  # 8
KC = NCHUNK * C                 # 128

LEVEL_N = [64, 32, 16, 8, 4, 2]
B_OFF = [0, 64, 96, 112, 120, 124]
NLOG = 4


def _bitrev(i, bits):
    r = 0
    for _ in range(bits):
        r = (r << 1) | (i & 1)
        i >>= 1
    return r


def _build_branch_vec(branch_lengths):
    bt = np.empty(126, np.float32)
    off_in = 0
    for k, n in enumerate(LEVEL_N):
        t = branch_lengths[off_in:off_in + n]
        off_in += n
        perm = [_bitrev(p, 6 - k) for p in range(n)]
        bt[B_OFF[k]:B_OFF[k] + n] = t[perm]
    return bt


def _pir_view(pir_chunk, n):
    """pir_chunk: [128, C, 4, 16] slice. View matching [C, 4, n] tmp layout."""
    if n > 16:
        r = n // 16
        return pir_chunk.unsqueeze(3).broadcast_to([128, C, NST, r, 16])
    return pir_chunk[:, :, :, 0:n]


def _tree(nc, pool, pir_chunk, eq, gap3, beta, logbuf, ll):
    """One Felsenstein tree. eq: [128, C*4*64] bf16 one-hot planes (gap rows
    all-zero), gap3: [128, C, 64] bf16 view, beta [128,126] bf16,
    logbuf [128, C*NLOG] f32, ll out [128, C] f32."""
    Lc = eq
    for k in range(6):
        n = LEVEL_N[k]
        nn = n // 2
        L4 = Lc[:].rearrange("p (c s n) -> p c s n", c=C, s=NST)
        tmp = pool.tile([128, C * 4 * n], BF16, tag="tmp", bufs=2)
        t4 = tmp[:].rearrange("p (c s n) -> p c s n", c=C, s=NST)
        if n > 16:
            nc.vector.tensor_mul(
                tmp[:].rearrange("p (c s r i) -> p c s r i", c=C, s=NST, i=16),
                Lc[:].rearrange("p (c s r i) -> p c s r i", c=C, s=NST, i=16),
                _pir_view(pir_chunk, n))
        else:
            nc.vector.tensor_mul(t4, L4, _pir_view(pir_chunk, n))
        dAB = pool.tile([128, C * 2 * n], BF16, tag="dAB", bufs=2)
        dAB4 = dAB[:].rearrange("p (c s n) -> p c s n", c=C, s=2)
        nc.vector.tensor_add(dAB4, t4[:, :, 0:2, :], t4[:, :, 2:4, :])
        dot = pool.tile([128, C * n], BF16, tag="dot", bufs=2)
        dot3 = dot[:].rearrange("p (c n) -> p c n", c=C)
        nc.vector.tensor_add(dot3, dAB4[:, :, 0, :], dAB4[:, :, 1, :])
        if k == 0:
            nc.vector.tensor_add(dot3, dot3, gap3)  # gap leaves: dot = 1
        q = pool.tile([128, C * n], BF16, tag="q", bufs=2)
        q3 = q[:].rearrange("p (c n) -> p c n", c=C)
        bk = beta[:][:, B_OFF[k]:B_OFF[k] + n].unsqueeze(1).broadcast_to([128, C, n])
        nc.vector.tensor_mul(q3, dot3, bk)
        if k == 0:
            nc.vector.tensor_add(q3, q3, gap3)  # gap leaves: w = 1 + beta
        w = pool.tile([128, C * 4 * n], BF16, tag="w", bufs=2)
        w4 = w[:].rearrange("p (c s n) -> p c s n", c=C, s=NST)
        nc.vector.tensor_add(w4, L4, q3.unsqueeze(2).broadcast_to([128, C, NST, n]))
        Lnew = pool.tile([128, C * 4 * nn], BF16, tag=f"L{k}", bufs=2)
        Ln4 = Lnew[:].rearrange("p (c s n) -> p c s n", c=C, s=NST)
        nc.vector.tensor_mul(Ln4, w4[:, :, :, 0:nn], w4[:, :, :, nn:n])
        Lc = Lnew
        if k == 3:  # adaptive rescale, nn = 4 nodes
            sAB = pool.tile([128, C * 2 * nn], F32, tag="sAB", bufs=2)
            sAB4 = sAB[:].rearrange("p (c s n) -> p c s n", c=C, s=2)
            nc.vector.tensor_add(sAB4, Ln4[:, :, 0:2, :], Ln4[:, :, 2:4, :])
            s = pool.tile([128, C * nn], F32, tag="s", bufs=2)
            s3 = s[:].rearrange("p (c n) -> p c n", c=C)
            nc.vector.tensor_add(s3, sAB4[:, :, 0, :], sAB4[:, :, 1, :])
            nc.vector.tensor_scalar_max(s[:], s[:], TINY)
            lb3 = logbuf[:].rearrange("p (c n) -> p c n", c=C)
            nc.scalar.activation(lb3, s3, AF.Ln)
            r = pool.tile([128, C * nn], F32, tag="r", bufs=2)
            nc.vector.reciprocal(r[:], s[:])
            Lrs = pool.tile([128, C * 4 * nn], BF16, tag="Lr3", bufs=2)
            Lr4 = Lrs[:].rearrange("p (c s n) -> p c s n", c=C, s=NST)
            nc.vector.tensor_mul(
                Lr4, Ln4,
                r[:].rearrange("p (c n) -> p c n", c=C).unsqueeze(2).broadcast_to(
                    [128, C, NST, nn]))
            Lc = Lrs

    # root: rdot = sum_s L*pi (f32); ll = ln(rdot) + sum(logbuf)
    L4 = Lc[:].rearrange("p (c s n) -> p c s n", c=C, s=NST)
    rt = pool.tile([128, C * 4], BF16, tag="rt", bufs=2)
    rt3 = rt[:].rearrange("p (c s) -> p c s", c=C)
    nc.vector.tensor_mul(rt3, L4[:, :, :, 0], pir_chunk[:, :, :, 0])
    rdot = pool.tile([128, C], F32, tag="rdot", bufs=2)
    rdot3 = rdot[:].rearrange("p (c n) -> p c n", c=C)
    nc.vector.tensor_reduce(rdot3, rt3, axis=AX.X, op=OP.add)
    nc.vector.tensor_scalar_max(rdot[:], rdot[:], TINY)
    lr = pool.tile([128, C], F32, tag="lr", bufs=2)
    nc.scalar.activation(lr[:], rdot[:], AF.Ln)
    lsum = pool.tile([128, C], F32, tag="lsum", bufs=2)
    nc.vector.tensor_reduce(
        lsum[:].rearrange("p c -> p c", c=C),
        logbuf[:].rearrange("p (c n) -> p c n", c=C), axis=AX.X, op=OP.add)
    nc.vector.tensor_add(ll[:], lr[:], lsum[:])


def _build_program():
    nc = bacc.Bacc("TRN2", target_bir_lowering=False, debug=False,
                   num_devices=N_CORES)
    cod_d = nc.dram_tensor("cod", [CORE_SITES, S], BF16, kind="ExternalInput")
    lg_d = nc.dram_tensor("lg", [CORE_SITES, NST], F32, kind="ExternalInput")
    pt_d = nc.dram_tensor("pt", [CORE_SITES, NST], F32, kind="ExternalInput")
    mk_d = nc.dram_tensor("mk", [CORE_SITES], F32, kind="ExternalInput")
    bt_d = nc.dram_tensor("bt", [126], F32, kind="ExternalInput")
    po_d = nc.dram_tensor("po", [128, 2], F32, kind="ExternalOutput")

    cod_r = cod_d.ap().rearrange("(k p c) l -> k p (c l)", k=NCHUNK, p=128)

    with tile.TileContext(nc) as tc:
        with (tc.tile_pool(name="const", bufs=1) as cpool,
              tc.tile_pool(name="work", bufs=2) as pool):
            # ---------- upfront: constants + all Exp/Copy ACT work ----------
            btf = cpool.tile([128, 126], F32)
            nc.sync.dma_start(btf[:], bt_d.ap().unsqueeze(0).broadcast_to([128, 126]))
            bte = cpool.tile([128, 126], F32)
            nc.scalar.activation(bte[:], btf[:], AF.Exp, scale=MU)
            beta = cpool.tile([128, 126], BF16)
            nc.vector.tensor_scalar(beta[:], bte[:], -1.0, None, OP.add)
            acc = cpool.tile([128, 1], F32)
            nc.vector.memset(acc[:], 0.0)
            cnt = cpool.tile([128, 1], F32)
            nc.vector.memset(cnt[:], 0.0)

            lga = cpool.tile([128, KC * NST], F32)
            nc.sync.dma_start(
                lga[:].rearrange("p (k x) -> p k x", k=NCHUNK),
                lg_d.ap().rearrange("(k p c) s -> k p (c s)",
                                    k=NCHUNK, p=128).transpose([1, 0, 2]))
            pta = cpool.tile([128, KC * NST], F32)
            nc.sync.dma_start(
                pta[:].rearrange("p (k x) -> p k x", k=NCHUNK),
                pt_d.ap().rearrange("(k p c) s -> k p (c s)",
                                    k=NCHUNK, p=128).transpose([1, 0, 2]))
            mka = cpool.tile([128, KC], F32)
            nc.sync.dma_start(
                mka[:].rearrange("p (k c) -> p k c", k=NCHUNK),
                mk_d.ap().rearrange("(k p c) -> k p c",
                                    k=NCHUNK, p=128).transpose([1, 0, 2]))

            # softmax(logits) for all chunks (no max-sub: |logits| is small)
            exa = cpool.tile([128, KC * NST], F32)
            nc.scalar.activation(exa[:], lga[:], AF.Exp)
            sea = cpool.tile([128, KC], F32)
            nc.vector.tensor_reduce(
                sea[:].rearrange("p (g n) -> p g n", g=KC),
                exa[:].rearrange("p (g s) -> p g s", g=KC), axis=AX.X, op=OP.add)
            rea = cpool.tile([128, KC], F32)
            nc.vector.reciprocal(rea[:], sea[:])
            pipa = cpool.tile([128, KC * NST], F32)
            nc.vector.tensor_mul(
                pipa[:].rearrange("p (g s) -> p g s", g=KC),
                exa[:].rearrange("p (g s) -> p g s", g=KC),
                rea[:].rearrange("p (g n) -> p g n", g=KC).broadcast_to(
                    [128, KC, NST]))
            # pi_true clamp + normalize
            ptca = cpool.tile([128, KC * NST], F32)
            nc.vector.tensor_scalar_max(ptca[:], pta[:], EPS)
            sea2 = cpool.tile([128, KC], F32)
            nc.vector.tensor_reduce(
                sea2[:].rearrange("p (g n) -> p g n", g=KC),
                ptca[:].rearrange("p (g s) -> p g s", g=KC), axis=AX.X, op=OP.add)
            rea2 = cpool.tile([128, KC], F32)
            nc.vector.reciprocal(rea2[:], sea2[:])
            pita = cpool.tile([128, KC * NST], F32)
            nc.vector.tensor_mul(
                pita[:].rearrange("p (g s) -> p g s", g=KC),
                ptca[:].rearrange("p (g s) -> p g s", g=KC),
                rea2[:].rearrange("p (g n) -> p g n", g=KC).broadcast_to(
                    [128, KC, NST]))
            # materialize bf16 pi planes [g, s, 16] for both trees (ACT Copy)
            pra = cpool.tile([128, KC * NST * 16], BF16)
            nc.scalar.activation(
                pra[:].rearrange("p (g s i) -> p g s i", g=KC, s=NST),
                pipa[:].rearrange("p (g s) -> p g s", g=KC).unsqueeze(3)
                .broadcast_to([128, KC, NST, 16]), AF.Copy)
            prt = cpool.tile([128, KC * NST * 16], BF16)
            nc.scalar.activation(
                prt[:].rearrange("p (g s i) -> p g s i", g=KC, s=NST),
                pita[:].rearrange("p (g s) -> p g s", g=KC).unsqueeze(3)
                .broadcast_to([128, KC, NST, 16]), AF.Copy)
            pra5 = pra[:].rearrange("p (k c s i) -> p k c s i", k=NCHUNK, c=C, s=NST)
            prt5 = prt[:].rearrange("p (k c s i) -> p k c s i", k=NCHUNK, c=C, s=NST)
            mka3 = mka[:].rearrange("p (k c) -> p k c", k=NCHUNK)

            # ---------- chunk loop: DVE + Ln only ----------
            for kc in range(NCHUNK):
                cod = pool.tile([128, C * S], BF16, tag="cod", bufs=2)
                nc.sync.dma_start(cod[:], cod_r[kc])
                gap = pool.tile([128, C * S], BF16, tag="gap", bufs=2)
                nc.vector.tensor_scalar(gap[:], cod[:], 4.0, None, OP.is_ge)
                gap3 = gap[:].rearrange("p (c l) -> p c l", c=C)
                eq = pool.tile([128, C * NST * S], BF16, tag="eq", bufs=2)
                eq4 = eq[:].rearrange("p (c s l) -> p c s l", c=C, s=NST)
                cod3 = cod[:].rearrange("p (c l) -> p c l", c=C)
                for c in range(NST):
                    nc.vector.tensor_scalar(eq4[:, :, c, :], cod3, float(c),
                                            None, OP.is_equal)

                ll_p = pool.tile([128, C], F32, tag="ll_p", bufs=2)
                logbuf_p = pool.tile([128, C * NLOG], F32, tag="logbuf_p", bufs=2)
                _tree(nc, pool, pra5[:, kc], eq, gap3, beta, logbuf_p, ll_p)
                ll_t = pool.tile([128, C], F32, tag="ll_t", bufs=2)
                logbuf_t = pool.tile([128, C * NLOG], F32, tag="logbuf_t", bufs=2)
                _tree(nc, pool, prt5[:, kc], eq, gap3, beta, logbuf_t, ll_t)

                diff = pool.tile([128, C], F32, tag="diff", bufs=2)
                nc.vector.tensor_sub(diff[:], ll_t[:], ll_p[:])
                dm = pool.tile([128, C], F32, tag="dm", bufs=2)
                nc.vector.tensor_mul(dm[:], diff[:], mka3[:, kc])
                racc = pool.tile([128, 1], F32, tag="racc", bufs=2)
                nc.vector.tensor_reduce(racc[:], dm[:], axis=AX.X, op=OP.add)
                nc.vector.tensor_add(acc[:], acc[:], racc[:])

            rcnt = cpool.tile([128, 1], F32)
            nc.vector.tensor_reduce(rcnt[:], mka[:], axis=AX.X, op=OP.add)
            po = cpool.tile([128, 2], F32)
            nc.vector.tensor_copy(po[:, 0:1], acc[:])
            nc.vector.tensor_copy(po[:, 1:2], rcnt[:])
            nc.sync.dma_start(po_d.ap()[:], po[:])

    nc.compile()
    return nc


_NC_CACHE = None


def _get_nc():
    global _NC_CACHE
    if _NC_CACHE is None:
        _NC_CACHE = _build_program()
    return _NC_CACHE


def _prep_inputs(logits, msa_codes, pi_true, valid_mask, branch_lengths):
    logits = np.asarray(logits)
    msa_codes = np.asarray(msa_codes)
    pi_true = np.asarray(pi_true)
    valid_mask = np.asarray(valid_mask)
    branch_lengths = np.asarray(branch_lengths)
    perm = np.array([_bitrev(p, 6) for p in range(S)])
    cod = np.ascontiguousarray(
        msa_codes.reshape(SITES, S)[:, perm]).astype(ml_dtypes.bfloat16)
    lg = np.ascontiguousarray(logits.reshape(SITES, NST)).astype(np.float32)
    pt = np.ascontiguousarray(pi_true.reshape(SITES, NST)).astype(np.float32)
    mk = valid_mask.reshape(SITES).astype(np.float32)
    bt = _build_branch_vec(np.asarray(branch_lengths, np.float32))
    in_maps = []
    for i in range(N_CORES):
        sl = slice(i * CORE_SITES, (i + 1) * CORE_SITES)
        in_maps.append(dict(cod=cod[sl], lg=lg[sl], pt=pt[sl], mk=mk[sl], bt=bt))
    return in_maps


def kernel(logits, msa_codes, pi_true, valid_mask, branch_lengths):
    nc = _get_nc()
    in_maps = _prep_inputs(logits, msa_codes, pi_true, valid_mask,
                           branch_lengths)
    res = run_bass_kernel_spmd(nc, in_maps, core_ids=list(range(N_CORES)))
    tot = 0.0
    n = 0.0
    for i in range(N_CORES):
        po = res.results[i]["po"]
        tot += float(po[:, 0].sum())
        n += float(po[:, 1].sum())
    return np.float32(tot / max(n, 1.0))


# revision 12
# speedup vs baseline: 275.2518x; 1.0973x over previous
"""F81 supervised loss (Felsenstein pruning, balanced 64-leaf tree) on 8 TRN2 cores.

v5: bf16 plane layout [group, state, node], bit-reversed leaves (merges =
contiguous half-products in DVE 2x mode). Algebra: w = L + beta*dot with
beta=(1-a)/a; all per-node a-factors become per-tree constants that cancel
in ll_true - ll_pred, so no per-level normalization is needed -- one
adaptive rescale at level 3 keeps bf16 in range. All Exp/Copy activation
work is batched upfront so the chunk loop only uses Ln (single ACT table
set, no reload thrash). Gap leaves handled in closed form. Both trees
(pi_pred / pi_true) run fused through one set of double-width ops
(G = 2C site-groups), halving instruction count.
"""

import numpy as np
import ml_dtypes
import concourse.bass as bass  # noqa: F401
import concourse.tile as tile
from concourse import bacc, mybir
from concourse.bass_utils import run_bass_kernel_spmd

F32 = mybir.dt.float32
BF16 = mybir.dt.bfloat16
OP = mybir.AluOpType
AF = mybir.ActivationFunctionType
AX = mybir.AxisListType

N_CORES = 8
B, L, S = 16, 8192, 64
NST = 4
MU = 1.0
EPS = 1e-12
TINY = 1e-35
SITES = B * L
CORE_SITES = SITES // N_CORES   # 16384
C = 16                          # site-groups per chunk (per tree)
G = 2 * C                       # fused groups: [0:C]=pred, [C:2C]=true
CHUNK_SITES = 128 * C           # 2048
NCHUNK = CORE_SITES // CHUNK_SITES  # 8
KC = NCHUNK * C                 # 128

LEVEL_N = [64, 32, 16, 8, 4, 2]
B_OFF = [0, 64, 96, 112, 120, 124]
NLOG = 4


def _bitrev(i, bits):
    r = 0
    for _ in range(bits):
        r = (r << 1) | (i & 1)
        i >>= 1
    return r


def _build_branch_vec(branch_lengths):
    bt = np.empty(126, np.float32)
    off_in = 0
    for k, n in enumerate(LEVEL_N):
        t = branch_lengths[off_in:off_in + n]
        off_in += n
        perm = [_bitrev(p, 6 - k) for p in range(n)]
        bt[B_OFF[k]:B_OFF[k] + n] = t[perm]
    return bt


def _pir_view(pir_chunk, n):
    """pir_chunk: [128, G, 4, 16]. View matching [G, 4, n] plane tmp layout."""
    if n > 16:
        r = n // 16
        return pir_chunk.unsqueeze(3).broadcast_to([128, G, NST, r, 16])
    return pir_chunk[:, :, :, 0:n]


def _tree_pair(nc, pool, pir_chunk, eq, gap3, beta, logbuf, ll):
    """Both Felsenstein trees fused over G=2C groups. eq: [128, G*4*64] bf16
    one-hot planes duplicated per tree (gap rows all-zero), gap3:
    [128, C, 64] bf16 view, beta [128,126] bf16, logbuf [128, G*NLOG] f32,
    ll out [128, G] f32."""
    Lc = None
    for k in range(6):
        n = LEVEL_N[k]
        nn = n // 2
        tmp = pool.tile([128, G * 4 * n], BF16, tag="tmp", bufs=1)
        t4 = tmp[:].rearrange("p (c s n) -> p c s n", c=G, s=NST)
        if k == 0:
            # level 0 reads the C-wide one-hot tile once per tree
            eq5 = eq[:].rearrange("p (c s r i) -> p c s r i", c=C, s=NST, i=16)
            tmp6 = tmp[:].rearrange("p (t c s r i) -> p t c s r i",
                                    t=2, c=C, s=NST, i=16)
            for t in range(2):
                nc.vector.tensor_mul(
                    tmp6[:, t], eq5,
                    pir_chunk[:, t * C:(t + 1) * C].unsqueeze(3)
                    .broadcast_to([128, C, NST, n // 16, 16]))
        elif n > 16:
            nc.vector.tensor_mul(
                tmp[:].rearrange("p (c s r i) -> p c s r i", c=G, s=NST, i=16),
                Lc[:].rearrange("p (c s r i) -> p c s r i", c=G, s=NST, i=16),
                _pir_view(pir_chunk, n))
        else:
            L4 = Lc[:].rearrange("p (c s n) -> p c s n", c=G, s=NST)
            nc.vector.tensor_mul(t4, L4, _pir_view(pir_chunk, n))
        dAB = pool.tile([128, G * 2 * n], BF16, tag="dAB", bufs=1)
        dAB4 = dAB[:].rearrange("p (c s n) -> p c s n", c=G, s=2)
        nc.vector.tensor_add(dAB4, t4[:, :, 0:2, :], t4[:, :, 2:4, :])
        dot = pool.tile([128, G * n], BF16, tag="dot", bufs=1)
        dot3 = dot[:].rearrange("p (c n) -> p c n", c=G)
        nc.vector.tensor_add(dot3, dAB4[:, :, 0, :], dAB4[:, :, 1, :])
        if k == 0:
            gb = gap3.unsqueeze(1).broadcast_to([128, 2, C, n])
            dot4t = dot[:].rearrange("p (t c n) -> p t c n", t=2, c=C)
            nc.vector.tensor_add(dot4t, dot4t, gb)  # gap leaves: dot = 1
        q = pool.tile([128, G * n], BF16, tag="q", bufs=1)
        q3 = q[:].rearrange("p (c n) -> p c n", c=G)
        bk = beta[:][:, B_OFF[k]:B_OFF[k] + n].unsqueeze(1).broadcast_to([128, G, n])
        nc.vector.tensor_mul(q3, dot3, bk)
        if k == 0:
            gb = gap3.unsqueeze(1).broadcast_to([128, 2, C, n])
            q4t = q[:].rearrange("p (t c n) -> p t c n", t=2, c=C)
            nc.vector.tensor_add(q4t, q4t, gb)  # gap leaves: w = 1 + beta
        w = pool.tile([128, G * 4 * n], BF16, tag="w", bufs=1)
        w4 = w[:].rearrange("p (c s n) -> p c s n", c=G, s=NST)
        if k == 0:
            eq4c = eq[:].rearrange("p (c s n) -> p c s n", c=C, s=NST)
            w5 = w[:].rearrange("p (t c s n) -> p t c s n", t=2, c=C, s=NST)
            q4 = q[:].rearrange("p (t c n) -> p t c n", t=2, c=C)
            for t in range(2):
                nc.vector.tensor_add(
                    w5[:, t], eq4c,
                    q4[:, t].unsqueeze(2).broadcast_to([128, C, NST, n]))
        else:
            L4 = Lc[:].rearrange("p (c s n) -> p c s n", c=G, s=NST)
            nc.vector.tensor_add(
                w4, L4, q3.unsqueeze(2).broadcast_to([128, G, NST, n]))
        Lnew = pool.tile([128, G * 4 * nn], BF16, tag=f"L{k}", bufs=1)
        Ln4 = Lnew[:].rearrange("p (c s n) -> p c s n", c=G, s=NST)
        nc.vector.tensor_mul(Ln4, w4[:, :, :, 0:nn], w4[:, :, :, nn:n])
        Lc = Lnew
        if k == 3:  # adaptive rescale, nn = 4 nodes
            sAB = pool.tile([128, G * 2 * nn], F32, tag="sAB", bufs=2)
            sAB4 = sAB[:].rearrange("p (c s n) -> p c s n", c=G, s=2)
            nc.vector.tensor_add(sAB4, Ln4[:, :, 0:2, :], Ln4[:, :, 2:4, :])
            s = pool.tile([128, G * nn], F32, tag="s", bufs=2)
            s3 = s[:].rearrange("p (c n) -> p c n", c=G)
            nc.vector.tensor_add(s3, sAB4[:, :, 0, :], sAB4[:, :, 1, :])
            nc.vector.tensor_scalar_max(s[:], s[:], TINY)
            lb3 = logbuf[:].rearrange("p (c n) -> p c n", c=G)
            nc.scalar.activation(lb3, s3, AF.Ln)
            r = pool.tile([128, G * nn], F32, tag="r", bufs=2)
            nc.vector.reciprocal(r[:], s[:])
            Lrs = pool.tile([128, G * 4 * nn], BF16, tag="Lr3", bufs=1)
            Lr4 = Lrs[:].rearrange("p (c s n) -> p c s n", c=G, s=NST)
            nc.vector.tensor_mul(
                Lr4, Ln4,
                r[:].rearrange("p (c n) -> p c n", c=G).unsqueeze(2).broadcast_to(
                    [128, G, NST, nn]))
            Lc = Lrs

    # root: rdot = sum_s L*pi (f32); ll = ln(rdot) + sum(logbuf)
    L4 = Lc[:].rearrange("p (c s n) -> p c s n", c=G, s=NST)
    rt = pool.tile([128, G * 4], BF16, tag="rt", bufs=2)
    rt3 = rt[:].rearrange("p (c s) -> p c s", c=G)
    nc.vector.tensor_mul(rt3, L4[:, :, :, 0], pir_chunk[:, :, :, 0])
    rdot = pool.tile([128, G], F32, tag="rdot", bufs=2)
    rdot3 = rdot[:].rearrange("p (c n) -> p c n", c=G)
    nc.vector.tensor_reduce(rdot3, rt3, axis=AX.X, op=OP.add)
    nc.vector.tensor_scalar_max(rdot[:], rdot[:], TINY)
    lr = pool.tile([128, G], F32, tag="lr", bufs=2)
    nc.scalar.activation(lr[:], rdot[:], AF.Ln)
    lsum = pool.tile([128, G], F32, tag="lsum", bufs=2)
    nc.vector.tensor_reduce(
        lsum[:].rearrange("p c -> p c", c=G),
        logbuf[:].rearrange("p (c n) -> p c n", c=G), axis=AX.X, op=OP.add)
    nc.vector.tensor_add(ll[:], lr[:], lsum[:])


def _build_program():
    nc = bacc.Bacc("TRN2", target_bir_lowering=False, debug=False,
                   num_devices=N_CORES)
    cod_d = nc.dram_tensor("cod", [CORE_SITES, S], BF16, kind="ExternalInput")
    lg_d = nc.dram_tensor("lg", [CORE_SITES, NST], F32, kind="ExternalInput")
    pt_d = nc.dram_tensor("pt", [CORE_SITES, NST], F32, kind="ExternalInput")
    mk_d = nc.dram_tensor("mk", [CORE_SITES], F32, kind="ExternalInput")
    bt_d = nc.dram_tensor("bt", [126], F32, kind="ExternalInput")
    po_d = nc.dram_tensor("po", [128, 2], F32, kind="ExternalOutput")

    cod_r = cod_d.ap().rearrange("(k p c) l -> k p (c l)", k=NCHUNK, p=128)

    with tile.TileContext(nc) as tc:
        with (tc.tile_pool(name="const", bufs=1) as cpool,
              tc.tile_pool(name="work", bufs=2) as pool):
            # ---------- upfront: constants + all Exp/Copy ACT work ----------
            btf = cpool.tile([128, 126], F32)
            nc.sync.dma_start(btf[:], bt_d.ap().unsqueeze(0).broadcast_to([128, 126]))
            bte = cpool.tile([128, 126], F32)
            nc.scalar.activation(bte[:], btf[:], AF.Exp, scale=MU)
            beta = cpool.tile([128, 126], BF16)
            nc.vector.tensor_scalar(beta[:], bte[:], -1.0, None, OP.add)
            acc = cpool.tile([128, 1], F32)
            nc.vector.memset(acc[:], 0.0)

            lga = cpool.tile([128, KC * NST], F32)
            nc.sync.dma_start(
                lga[:].rearrange("p (k x) -> p k x", k=NCHUNK),
                lg_d.ap().rearrange("(k p c) s -> k p (c s)",
                                    k=NCHUNK, p=128).transpose([1, 0, 2]))
            pta = cpool.tile([128, KC * NST], F32)
            nc.sync.dma_start(
                pta[:].rearrange("p (k x) -> p k x", k=NCHUNK),
                pt_d.ap().rearrange("(k p c) s -> k p (c s)",
                                    k=NCHUNK, p=128).transpose([1, 0, 2]))
            mka = cpool.tile([128, KC], F32)
            nc.sync.dma_start(
                mka[:].rearrange("p (k c) -> p k c", k=NCHUNK),
                mk_d.ap().rearrange("(k p c) -> k p c",
                                    k=NCHUNK, p=128).transpose([1, 0, 2]))

            # softmax(logits) for all chunks (no max-sub: |logits| is small)
            exa = cpool.tile([128, KC * NST], F32)
            nc.scalar.activation(exa[:], lga[:], AF.Exp)
            sea = cpool.tile([128, KC], F32)
            nc.vector.tensor_reduce(
                sea[:].rearrange("p (g n) -> p g n", g=KC),
                exa[:].rearrange("p (g s) -> p g s", g=KC), axis=AX.X, op=OP.add)
            rea = cpool.tile([128, KC], F32)
            nc.vector.reciprocal(rea[:], sea[:])
            pipa = cpool.tile([128, KC * NST], F32)
            nc.vector.tensor_mul(
                pipa[:].rearrange("p (g s) -> p g s", g=KC),
                exa[:].rearrange("p (g s) -> p g s", g=KC),
                rea[:].rearrange("p (g n) -> p g n", g=KC).broadcast_to(
                    [128, KC, NST]))
            # pi_true clamp + normalize
            ptca = cpool.tile([128, KC * NST], F32)
            nc.vector.tensor_scalar_max(ptca[:], pta[:], EPS)
            sea2 = cpool.tile([128, KC], F32)
            nc.vector.tensor_reduce(
                sea2[:].rearrange("p (g n) -> p g n", g=KC),
                ptca[:].rearrange("p (g s) -> p g s", g=KC), axis=AX.X, op=OP.add)
            rea2 = cpool.tile([128, KC], F32)
            nc.vector.reciprocal(rea2[:], sea2[:])
            pita = cpool.tile([128, KC * NST], F32)
            nc.vector.tensor_mul(
                pita[:].rearrange("p (g s) -> p g s", g=KC),
                ptca[:].rearrange("p (g s) -> p g s", g=KC),
                rea2[:].rearrange("p (g n) -> p g n", g=KC).broadcast_to(
                    [128, KC, NST]))
            # bf16 pi planes for both trees, interleaved per chunk:
            # prb layout [k, tree, c, s, i=16]
            prb = cpool.tile([128, NCHUNK * G * NST * 16], BF16)
            prb6 = prb[:].rearrange("p (k t c s i) -> p k t c s i",
                                    k=NCHUNK, t=2, c=C, s=NST)
            nc.scalar.activation(
                prb6[:, :, 0],
                pipa[:].rearrange("p (k c s) -> p k c s", k=NCHUNK, c=C)
                .unsqueeze(4).broadcast_to([128, NCHUNK, C, NST, 16]), AF.Copy)
            nc.scalar.activation(
                prb6[:, :, 1],
                pita[:].rearrange("p (k c s) -> p k c s", k=NCHUNK, c=C)
                .unsqueeze(4).broadcast_to([128, NCHUNK, C, NST, 16]), AF.Copy)
            prb5 = prb[:].rearrange("p (k g s i) -> p k g s i",
                                    k=NCHUNK, g=G, s=NST)
            mka3 = mka[:].rearrange("p (k c) -> p k c", k=NCHUNK)

            # ---------- chunk loop: DVE + Ln only ----------
            for kc in range(NCHUNK):
                cod = pool.tile([128, C * S], BF16, tag="cod", bufs=2)
                nc.sync.dma_start(cod[:], cod_r[kc])
                gap = pool.tile([128, C * S], BF16, tag="gap", bufs=2)
                nc.vector.tensor_scalar(gap[:], cod[:], 4.0, None, OP.is_ge)
                gap3 = gap[:].rearrange("p (c l) -> p c l", c=C)
                # one-hot planes (C-wide; level 0 reads it once per tree)
                eq = pool.tile([128, C * NST * S], BF16, tag="eq", bufs=1)
                eq4 = eq[:].rearrange("p (c s l) -> p c s l", c=C, s=NST)
                cod3 = cod[:].rearrange("p (c l) -> p c l", c=C)
                for c in range(NST):
                    nc.vector.tensor_scalar(eq4[:, :, c, :], cod3, float(c),
                                            None, OP.is_equal)

                ll = pool.tile([128, G], F32, tag="ll", bufs=2)
                logbuf = pool.tile([128, G * NLOG], F32, tag="logbuf", bufs=2)
                _tree_pair(nc, pool, prb5[:, kc], eq, gap3, beta, logbuf, ll)

                diff = pool.tile([128, C], F32, tag="diff", bufs=2)
                nc.vector.tensor_sub(diff[:], ll[:, C:G], ll[:, 0:C])
                dm = pool.tile([128, C], F32, tag="dm", bufs=2)
                nc.vector.tensor_mul(dm[:], diff[:], mka3[:, kc])
                racc = pool.tile([128, 1], F32, tag="racc", bufs=2)
                nc.vector.tensor_reduce(racc[:], dm[:], axis=AX.X, op=OP.add)
                nc.vector.tensor_add(acc[:], acc[:], racc[:])

            rcnt = cpool.tile([128, 1], F32)
            nc.vector.tensor_reduce(rcnt[:], mka[:], axis=AX.X, op=OP.add)
            po = cpool.tile([128, 2], F32)
            nc.vector.tensor_copy(po[:, 0:1], acc[:])
            nc.vector.tensor_copy(po[:, 1:2], rcnt[:])
            nc.sync.dma_start(po_d.ap()[:], po[:])

    nc.compile()
    return nc


_NC_CACHE = None


def _get_nc():
    global _NC_CACHE
    if _NC_CACHE is None:
        _NC_CACHE = _build_program()
    return _NC_CACHE


def _prep_inputs(logits, msa_codes, pi_true, valid_mask, branch_lengths):
    logits = np.asarray(logits)
    msa_codes = np.asarray(msa_codes)
    pi_true = np.asarray(pi_true)
    valid_mask = np.asarray(valid_mask)
    branch_lengths = np.asarray(branch_lengths)
    perm = np.array([_bitrev(p, 6) for p in range(S)])
    cod = np.ascontiguousarray(
        msa_codes.reshape(SITES, S)[:, perm]).astype(ml_dtypes.bfloat16)
    lg = np.ascontiguousarray(logits.reshape(SITES, NST)).astype(np.float32)
    pt = np.ascontiguousarray(pi_true.reshape(SITES, NST)).astype(np.float32)
    mk = valid_mask.reshape(SITES).astype(np.float32)
    bt = _build_branch_vec(np.asarray(branch_lengths, np.float32))
    in_maps = []
    for i in range(N_CORES):
        sl = slice(i * CORE_SITES, (i + 1) * CORE_SITES)
        in_maps.append(dict(cod=cod[sl], lg=lg[sl], pt=pt[sl], mk=mk[sl], bt=bt))
    return in_maps


def kernel(logits, msa_codes, pi_true, valid_mask, branch_lengths):
    nc = _get_nc()
    in_maps = _prep_inputs(logits, msa_codes, pi_true, valid_mask,
                           branch_lengths)
    res = run_bass_kernel_spmd(nc, in_maps, core_ids=list(range(N_CORES)))
    tot = 0.0
    n = 0.0
    for i in range(N_CORES):
        po = res.results[i]["po"]
        tot += float(po[:, 0].sum())
        n += float(po[:, 1].sum())
    return np.float32(tot / max(n, 1.0))


# revision 21
# speedup vs baseline: 434.5288x; 1.5787x over previous
"""F81 supervised loss (Felsenstein pruning, balanced 64-leaf tree) on 8 TRN2 cores.

v5: bf16 plane layout [group, state, node], bit-reversed leaves (merges =
contiguous half-products in DVE 2x mode). Algebra: w = L + beta*dot with
beta=(1-a)/a; all per-node a-factors become per-tree constants that cancel
in ll_true - ll_pred, so no per-level normalization is needed -- one
adaptive rescale at level 3 keeps bf16 in range. All Exp/Copy activation
work is batched upfront so the chunk loop only uses Ln (single ACT table
set, no reload thrash). Gap leaves handled in closed form. Both trees
(pi_pred / pi_true) run fused through one set of double-width ops
(G = 2C site-groups), halving instruction count.
"""

import numpy as np
import ml_dtypes
import concourse.bass as bass  # noqa: F401
import concourse.tile as tile
from concourse import bacc, mybir
from concourse.bass_utils import run_bass_kernel_spmd

F32 = mybir.dt.float32
BF16 = mybir.dt.bfloat16
OP = mybir.AluOpType
AF = mybir.ActivationFunctionType
AX = mybir.AxisListType

N_CORES = 8
B, L, S = 16, 8192, 64
NST = 4
MU = 1.0
EPS = 1e-12
TINY = 1e-35
SITES = B * L
CORE_SITES = SITES // N_CORES   # 16384
C = 16                          # site-groups per chunk (per tree)
G = 2 * C                       # fused groups: [0:C]=pred, [C:2C]=true
CHUNK_SITES = 128 * C           # 2048
NCHUNK = CORE_SITES // CHUNK_SITES  # 8
KC = NCHUNK * C                 # 128

LEVEL_N = [64, 32, 16, 8, 4, 2]
B_OFF = [0, 64, 96, 112, 120, 124]
NLOG = 4


def _bitrev(i, bits):
    r = 0
    for _ in range(bits):
        r = (r << 1) | (i & 1)
        i >>= 1
    return r


def _build_branch_vec(branch_lengths):
    bt = np.empty(126, np.float32)
    off_in = 0
    for k, n in enumerate(LEVEL_N):
        t = branch_lengths[off_in:off_in + n]
        off_in += n
        perm = [_bitrev(p, 6 - k) for p in range(n)]
        bt[B_OFF[k]:B_OFF[k] + n] = t[perm]
    return bt


def _pir_view(pir_chunk, n):
    """pir_chunk: [128, G, 4, 16]. View matching [G, 4, n] plane tmp layout."""
    if n > 16:
        r = n // 16
        return pir_chunk.unsqueeze(3).broadcast_to([128, G, NST, r, 16])
    return pir_chunk[:, :, :, 0:n]


def _tree_pair(nc, pool, pir_chunk, eq, gap3, beta, logbuf, ll):
    """Both Felsenstein trees fused over G=2C groups. eq: [128, G*4*64] bf16
    one-hot planes duplicated per tree (gap rows all-zero), gap3:
    [128, C, 64] bf16 view, beta [128,126] bf16, logbuf [128, G*NLOG] f32,
    ll out [128, G] f32."""
    Lc = None
    for k in range(6):
        n = LEVEL_N[k]
        nn = n // 2
        tmp = pool.tile([128, G * 4 * n], BF16, tag="tmp", bufs=1)
        t4 = tmp[:].rearrange("p (c s n) -> p c s n", c=G, s=NST)
        if k == 0:
            # level 0 reads the C-wide one-hot tile once per tree
            eq5 = eq[:].rearrange("p (c s r i) -> p c s r i", c=C, s=NST, i=16)
            tmp6 = tmp[:].rearrange("p (t c s r i) -> p t c s r i",
                                    t=2, c=C, s=NST, i=16)
            for t in range(2):
                nc.vector.tensor_mul(
                    tmp6[:, t], eq5,
                    pir_chunk[:, t * C:(t + 1) * C].unsqueeze(3)
                    .broadcast_to([128, C, NST, n // 16, 16]))
        elif n > 16:
            nc.vector.tensor_mul(
                tmp[:].rearrange("p (c s r i) -> p c s r i", c=G, s=NST, i=16),
                Lc[:].rearrange("p (c s r i) -> p c s r i", c=G, s=NST, i=16),
                _pir_view(pir_chunk, n))
        else:
            L4 = Lc[:].rearrange("p (c s n) -> p c s n", c=G, s=NST)
            nc.vector.tensor_mul(t4, L4, _pir_view(pir_chunk, n))
        dAB = pool.tile([128, G * 2 * n], BF16, tag="dAB", bufs=1)
        dAB4 = dAB[:].rearrange("p (c s n) -> p c s n", c=G, s=2)
        nc.vector.tensor_add(dAB4, t4[:, :, 0:2, :], t4[:, :, 2:4, :])
        dot = pool.tile([128, G * n], BF16, tag="dot", bufs=1)
        dot3 = dot[:].rearrange("p (c n) -> p c n", c=G)
        nc.vector.tensor_add(dot3, dAB4[:, :, 0, :], dAB4[:, :, 1, :])
        if k == 0:
            gb = gap3.unsqueeze(1).broadcast_to([128, 2, C, n])
            dot4t = dot[:].rearrange("p (t c n) -> p t c n", t=2, c=C)
            nc.vector.tensor_add(dot4t, dot4t, gb)  # gap leaves: dot = 1
        q = pool.tile([128, G * n], BF16, tag="q", bufs=1)
        q3 = q[:].rearrange("p (c n) -> p c n", c=G)
        bk = beta[:][:, B_OFF[k]:B_OFF[k] + n].unsqueeze(1).broadcast_to([128, G, n])
        nc.vector.tensor_mul(q3, dot3, bk)
        if k == 0:
            gb = gap3.unsqueeze(1).broadcast_to([128, 2, C, n])
            q4t = q[:].rearrange("p (t c n) -> p t c n", t=2, c=C)
            nc.vector.tensor_add(q4t, q4t, gb)  # gap leaves: w = 1 + beta
        w = pool.tile([128, G * 4 * n], BF16, tag="w", bufs=1)
        w4 = w[:].rearrange("p (c s n) -> p c s n", c=G, s=NST)
        if k == 0:
            eq4c = eq[:].rearrange("p (c s n) -> p c s n", c=C, s=NST)
            w5 = w[:].rearrange("p (t c s n) -> p t c s n", t=2, c=C, s=NST)
            q4 = q[:].rearrange("p (t c n) -> p t c n", t=2, c=C)
            for t in range(2):
                nc.vector.tensor_add(
                    w5[:, t], eq4c,
                    q4[:, t].unsqueeze(2).broadcast_to([128, C, NST, n]))
        else:
            L4 = Lc[:].rearrange("p (c s n) -> p c s n", c=G, s=NST)
            nc.vector.tensor_add(
                w4, L4, q3.unsqueeze(2).broadcast_to([128, G, NST, n]))
        Lnew = pool.tile([128, G * 4 * nn], BF16, tag=f"L{k}", bufs=1)
        Ln4 = Lnew[:].rearrange("p (c s n) -> p c s n", c=G, s=NST)
        nc.vector.tensor_mul(Ln4, w4[:, :, :, 0:nn], w4[:, :, :, nn:n])
        Lc = Lnew
        if k == 3:  # adaptive rescale, nn = 4 nodes
            sAB = pool.tile([128, G * 2 * nn], F32, tag="sAB", bufs=2)
            sAB4 = sAB[:].rearrange("p (c s n) -> p c s n", c=G, s=2)
            nc.vector.tensor_add(sAB4, Ln4[:, :, 0:2, :], Ln4[:, :, 2:4, :])
            s = pool.tile([128, G * nn], F32, tag="s", bufs=2)
            s3 = s[:].rearrange("p (c n) -> p c n", c=G)
            nc.vector.tensor_add(s3, sAB4[:, :, 0, :], sAB4[:, :, 1, :])
            nc.vector.tensor_scalar_max(s[:], s[:], TINY)
            lb3 = logbuf[:].rearrange("p (c n) -> p c n", c=G)
            nc.scalar.activation(lb3, s3, AF.Ln)
            r = pool.tile([128, G * nn], F32, tag="r", bufs=2)
            nc.vector.reciprocal(r[:], s[:])
            Lrs = pool.tile([128, G * 4 * nn], BF16, tag="Lr3", bufs=1)
            Lr4 = Lrs[:].rearrange("p (c s n) -> p c s n", c=G, s=NST)
            nc.vector.tensor_mul(
                Lr4, Ln4,
                r[:].rearrange("p (c n) -> p c n", c=G).unsqueeze(2).broadcast_to(
                    [128, G, NST, nn]))
            Lc = Lrs

    # root: rdot = sum_s L*pi (f32); ll = ln(rdot) + sum(logbuf)
    L4 = Lc[:].rearrange("p (c s n) -> p c s n", c=G, s=NST)
    rt = pool.tile([128, G * 4], BF16, tag="rt", bufs=2)
    rt3 = rt[:].rearrange("p (c s) -> p c s", c=G)
    nc.vector.tensor_mul(rt3, L4[:, :, :, 0], pir_chunk[:, :, :, 0])
    rdot = pool.tile([128, G], F32, tag="rdot", bufs=2)
    rdot3 = rdot[:].rearrange("p (c n) -> p c n", c=G)
    nc.vector.tensor_reduce(rdot3, rt3, axis=AX.X, op=OP.add)
    nc.vector.tensor_scalar_max(rdot[:], rdot[:], TINY)
    lr = pool.tile([128, G], F32, tag="lr", bufs=2)
    nc.scalar.activation(lr[:], rdot[:], AF.Ln)
    lsum = pool.tile([128, G], F32, tag="lsum", bufs=2)
    nc.vector.tensor_reduce(
        lsum[:].rearrange("p c -> p c", c=G),
        logbuf[:].rearrange("p (c n) -> p c n", c=G), axis=AX.X, op=OP.add)
    nc.vector.tensor_add(ll[:], lr[:], lsum[:])


def _build_program(nch):
    nc = bacc.Bacc("TRN2", target_bir_lowering=False, debug=False,
                   num_devices=N_CORES)
    ncs = nch * CHUNK_SITES
    kcv = nch * C
    cod_d = nc.dram_tensor("cod", [ncs, S], BF16, kind="ExternalInput")
    lg_d = nc.dram_tensor("lg", [ncs, NST], F32, kind="ExternalInput")
    pt_d = nc.dram_tensor("pt", [ncs, NST], F32, kind="ExternalInput")
    mk_d = nc.dram_tensor("mk", [ncs], F32, kind="ExternalInput")
    bt_d = nc.dram_tensor("bt", [126], F32, kind="ExternalInput")
    po_d = nc.dram_tensor("po", [128, 2], F32, kind="ExternalOutput")

    cod_r = cod_d.ap().rearrange("(k p c) l -> k p (c l)", k=nch, p=128)

    with tile.TileContext(nc) as tc:
        with (tc.tile_pool(name="const", bufs=1) as cpool,
              tc.tile_pool(name="work", bufs=2) as pool):
            # ---------- upfront: constants + all Exp/Copy ACT work ----------
            btf = cpool.tile([128, 126], F32)
            nc.sync.dma_start(btf[:], bt_d.ap().unsqueeze(0).broadcast_to([128, 126]))
            bte = cpool.tile([128, 126], F32)
            nc.scalar.activation(bte[:], btf[:], AF.Exp, scale=MU)
            beta = cpool.tile([128, 126], BF16)
            nc.vector.tensor_scalar(beta[:], bte[:], -1.0, None, OP.add)
            acc = cpool.tile([128, 1], F32)
            nc.vector.memset(acc[:], 0.0)

            lga = cpool.tile([128, kcv * NST], F32)
            nc.sync.dma_start(
                lga[:].rearrange("p (k x) -> p k x", k=nch),
                lg_d.ap().rearrange("(k p c) s -> k p (c s)",
                                    k=nch, p=128).transpose([1, 0, 2]))
            pta = cpool.tile([128, kcv * NST], F32)
            nc.sync.dma_start(
                pta[:].rearrange("p (k x) -> p k x", k=nch),
                pt_d.ap().rearrange("(k p c) s -> k p (c s)",
                                    k=nch, p=128).transpose([1, 0, 2]))
            mka = cpool.tile([128, kcv], F32)
            nc.sync.dma_start(
                mka[:].rearrange("p (k c) -> p k c", k=nch),
                mk_d.ap().rearrange("(k p c) -> k p c",
                                    k=nch, p=128).transpose([1, 0, 2]))

            # softmax(logits) for all chunks (no max-sub: |logits| is small)
            exa = cpool.tile([128, kcv * NST], F32)
            nc.scalar.activation(exa[:], lga[:], AF.Exp)
            sea = cpool.tile([128, kcv], F32)
            nc.vector.tensor_reduce(
                sea[:].rearrange("p (g n) -> p g n", g=kcv),
                exa[:].rearrange("p (g s) -> p g s", g=kcv), axis=AX.X, op=OP.add)
            rea = cpool.tile([128, kcv], F32)
            nc.vector.reciprocal(rea[:], sea[:])
            pipa = cpool.tile([128, kcv * NST], F32)
            nc.vector.tensor_mul(
                pipa[:].rearrange("p (g s) -> p g s", g=kcv),
                exa[:].rearrange("p (g s) -> p g s", g=kcv),
                rea[:].rearrange("p (g n) -> p g n", g=kcv).broadcast_to(
                    [128, kcv, NST]))
            # pi_true clamp + normalize
            ptca = cpool.tile([128, kcv * NST], F32)
            nc.vector.tensor_scalar_max(ptca[:], pta[:], EPS)
            sea2 = cpool.tile([128, kcv], F32)
            nc.vector.tensor_reduce(
                sea2[:].rearrange("p (g n) -> p g n", g=kcv),
                ptca[:].rearrange("p (g s) -> p g s", g=kcv), axis=AX.X, op=OP.add)
            rea2 = cpool.tile([128, kcv], F32)
            nc.vector.reciprocal(rea2[:], sea2[:])
            pita = cpool.tile([128, kcv * NST], F32)
            nc.vector.tensor_mul(
                pita[:].rearrange("p (g s) -> p g s", g=kcv),
                ptca[:].rearrange("p (g s) -> p g s", g=kcv),
                rea2[:].rearrange("p (g n) -> p g n", g=kcv).broadcast_to(
                    [128, kcv, NST]))
            # bf16 pi planes for both trees, interleaved per chunk:
            # prb layout [k, tree, c, s, i=16]
            prb = cpool.tile([128, nch * G * NST * 16], BF16)
            prb6 = prb[:].rearrange("p (k t c s i) -> p k t c s i",
                                    k=nch, t=2, c=C, s=NST)
            nc.scalar.activation(
                prb6[:, :, 0],
                pipa[:].rearrange("p (k c s) -> p k c s", k=nch, c=C)
                .unsqueeze(4).broadcast_to([128, nch, C, NST, 16]), AF.Copy)
            nc.scalar.activation(
                prb6[:, :, 1],
                pita[:].rearrange("p (k c s) -> p k c s", k=nch, c=C)
                .unsqueeze(4).broadcast_to([128, nch, C, NST, 16]), AF.Copy)
            prb5 = prb[:].rearrange("p (k g s i) -> p k g s i",
                                    k=nch, g=G, s=NST)
            mka3 = mka[:].rearrange("p (k c) -> p k c", k=nch)

            # ---------- chunk loop: DVE + Ln only ----------
            for kc in range(nch):
                cod = pool.tile([128, C * S], BF16, tag="cod", bufs=2)
                nc.sync.dma_start(cod[:], cod_r[kc])
                gap = pool.tile([128, C * S], BF16, tag="gap", bufs=2)
                nc.vector.tensor_scalar(gap[:], cod[:], 4.0, None, OP.is_ge)
                gap3 = gap[:].rearrange("p (c l) -> p c l", c=C)
                # one-hot planes (C-wide; level 0 reads it once per tree)
                eq = pool.tile([128, C * NST * S], BF16, tag="eq", bufs=1)
                eq4 = eq[:].rearrange("p (c s l) -> p c s l", c=C, s=NST)
                cod3 = cod[:].rearrange("p (c l) -> p c l", c=C)
                for c in range(NST):
                    nc.vector.tensor_scalar(eq4[:, :, c, :], cod3, float(c),
                                            None, OP.is_equal)

                ll = pool.tile([128, G], F32, tag="ll", bufs=2)
                logbuf = pool.tile([128, G * NLOG], F32, tag="logbuf", bufs=2)
                _tree_pair(nc, pool, prb5[:, kc], eq, gap3, beta, logbuf, ll)

                diff = pool.tile([128, C], F32, tag="diff", bufs=2)
                nc.vector.tensor_sub(diff[:], ll[:, C:G], ll[:, 0:C])
                dm = pool.tile([128, C], F32, tag="dm", bufs=2)
                nc.vector.tensor_mul(dm[:], diff[:], mka3[:, kc])
                racc = pool.tile([128, 1], F32, tag="racc", bufs=2)
                nc.vector.tensor_reduce(racc[:], dm[:], axis=AX.X, op=OP.add)
                nc.vector.tensor_add(acc[:], acc[:], racc[:])

            rcnt = cpool.tile([128, 1], F32)
            nc.vector.tensor_reduce(rcnt[:], mka[:], axis=AX.X, op=OP.add)
            po = cpool.tile([128, 2], F32)
            nc.vector.tensor_copy(po[:, 0:1], acc[:])
            nc.vector.tensor_copy(po[:, 1:2], rcnt[:])
            nc.sync.dma_start(po_d.ap()[:], po[:])

    nc.compile()
    return nc


_NC_CACHE = {}


def _get_nc(nch=NCHUNK):
    if nch not in _NC_CACHE:
        _NC_CACHE[nch] = _build_program(nch)
    return _NC_CACHE[nch]


def _prep_inputs(logits, msa_codes, pi_true, valid_mask, branch_lengths):
    """Compact to valid sites when they fit the fast (5-chunk) program;
    masked-out sites contribute nothing to the loss, so they need not be
    computed. Falls back to the full 8-chunk program otherwise."""
    logits = np.asarray(logits)
    msa_codes = np.asarray(msa_codes)
    pi_true = np.asarray(pi_true)
    valid_mask = np.asarray(valid_mask)
    branch_lengths = np.asarray(branch_lengths)
    mk_full = valid_mask.reshape(SITES).astype(np.float32)
    n_valid = int(mk_full.sum())
    # minimal program that holds all valid sites (capacity quantum =
    # N_CORES * CHUNK_SITES = 16384 sites)
    nch = min(NCHUNK, max(1, -(-n_valid // (N_CORES * CHUNK_SITES))))
    cap = N_CORES * nch * CHUNK_SITES
    if n_valid < SITES:
        idx = np.flatnonzero(mk_full)
        sites_idx = np.concatenate(
            [idx, np.zeros(cap - len(idx), np.int64)])
        mk = np.concatenate([np.ones(len(idx), np.float32),
                             np.zeros(cap - len(idx), np.float32)])
    else:
        sites_idx = np.arange(SITES)
        mk = mk_full
    perm = np.array([_bitrev(p, 6) for p in range(S)])
    cod = np.ascontiguousarray(
        msa_codes.reshape(SITES, S)[sites_idx][:, perm]).astype(
            ml_dtypes.bfloat16)
    lg = np.ascontiguousarray(
        logits.reshape(SITES, NST)[sites_idx]).astype(np.float32)
    pt = np.ascontiguousarray(
        pi_true.reshape(SITES, NST)[sites_idx]).astype(np.float32)
    bt = _build_branch_vec(np.asarray(branch_lengths, np.float32))
    in_maps = []
    per_core = nch * CHUNK_SITES
    for i in range(N_CORES):
        sl = slice(i * per_core, (i + 1) * per_core)
        in_maps.append(dict(cod=cod[sl], lg=lg[sl], pt=pt[sl], mk=mk[sl], bt=bt))
    return nch, in_maps


def kernel(logits, msa_codes, pi_true, valid_mask, branch_lengths):
    nch, in_maps = _prep_inputs(logits, msa_codes, pi_true, valid_mask,
                                branch_lengths)
    nc = _get_nc(nch)
    res = run_bass_kernel_spmd(nc, in_maps, core_ids=list(range(N_CORES)))
    tot = 0.0
    n = 0.0
    for i in range(N_CORES):
        po = res.results[i]["po"]
        tot += float(po[:, 0].sum())
        n += float(po[:, 1].sum())
    return np.float32(tot / max(n, 1.0))
